# revision 47
# baseline (speedup 1.0000x reference)
"""MaxUnpooling2D scatter kernel for Trainium2 (8 NeuronCores, batch-sharded).

Problem: x [16,64,64,128] f32, index [16,64,64,128] int64 (max-pool-argmax style
flat indices into the [16,128,128,128] output). Each pooled element (b,h,w,c)
scatters to ((b*128 + 2h+dh)*128 + 2w+dw)*128 + c with dh,dw in {0,1},
collision-free. Since C = 128 = 2^7 and 2W = 128 = 2^7:
    dw = bit 7 of index, dh = bit 14 of index
so the scatter is an elementwise masked interleave: for each of the 4 output
cells (dh,dw) of a 2x2 block, out = (index-bits == (dh,dw)) * x, written with a
strided access pattern. No on-device scatter needed, no cross-core traffic.

Sharding: batch dim across 8 cores (2 batch elements each). The host ships x
(f32) and a 2-bit cell code koff = 2*dh + dw, 4 codes per byte in a transposed
packing (IDX_MODE="pk2t": byte 4g+d holds codes c=16g+4y+d at bit-pair y).
The device unpacks with 4 uint32 ops (pk32 >> 2y) & 0x03030303 — each emits 4
codes as a natural-order uint32 write — then emits each output plane with one
fused (koff == k) * x scalar_tensor_tensor op per (dh,dw).

Per-core tiling: x[b] viewed as [128 partitions, 2048] where partition
p = (h_local, s) covers h = h0 + h_local (h0 in {0,32}), w in [16s, 16s+16).
Output tile [128, 8192] with per-partition free layout (t=dh, wl, dw, c);
two DMAs per tile (one per t) write it to DRAM as out[b, 2h+t, 32s+2wl+dw, c].
Input DMAs ride the ACT HWDGE ring, output DMAs the SP ring.

Measured (in-NEFF repetition slope, 8 cores): ~50 us/iteration, ~21.2 MB
HBM traffic per core (output-write dominated).
"""

import sys

import numpy as np

if "/opt/trn_rl_repo" not in sys.path:
    sys.path.insert(0, "/opt/trn_rl_repo")

B, H, W, C = 16, 64, 64, 128
N_CORES = 8
BPC = B // N_CORES  # batch elements per core
S = 4               # w-splits: partition covers W//S = 16 w values
HC = 128 // S       # 32 h rows per tile
WL = W // S         # 16
F = WL * C          # 2048 free elements per partition (input side)
TILES_PER_B = H // HC  # 2
N_TILES = BPC * TILES_PER_B  # 8

# Engine assignment for the 4 plane ops (dh, dw) -> "v" (DVE) or "g" (GPSIMD).
PLANE_ENGINES = {(0, 0): "v", (0, 1): "v", (1, 0): "v", (1, 1): "v"}
AND_ENGINE = "v"
# Index encoding shipped to the device:
#   "i16": low 16 bits of the flat index (device ANDs out bits 7/14)
#   "i8":  2-bit cell code koff = 2*dh + dw as int8 (host-decoded)
#   "pk2": koff packed 4-per-byte along c; device unpacks with 4 fused ts ops
#   "pk2t": transposed packing — byte 4g+d holds codes c=16g+4y+d at bits 2y,
#           so each uint32 op (pk32 >> 2y) & 0x03030303 emits 4 codes as one
#           natural-order uint32 write (decode FD drops 4x vs pk2)
IDX_MODE = "pk2t"

_CACHE: dict = {}


def build_program(
    reps: int = 1,
    variant: str = "full",
    in_eng: str = "scalar",
    op_bufs: int = 3,
    s_split: int = S,
    out_split: bool = False,
    gps_planes: int = 0,
    mask_dt: str = "bf16",
    io_bufs: int = 3,
    alt_rings: bool = False,
    dec16: bool = False,
    layout: str = "hs",
):
    """variant: 'full' | 'noand' | 'two_planes' | 'nodve' | 'nooutdma' | 'noindma'
    — non-'full' variants are timing probes only (wrong results).
    in_eng: which HWDGE ring issues input DMAs ('sync' or 'scalar' — outputs
    always go on 'sync', so 'scalar' gives inputs their own ring).
    out_split: issue the two per-tile output DMAs on different rings."""
    import concourse.mybir as mybir
    from concourse import bacc, tile

    S_, HC_, WL_ = s_split, 128 // s_split, W // s_split
    F_ = WL_ * C
    TILES_PER_B_ = H // HC_
    N_TILES_ = BPC * TILES_PER_B_

    nc = bacc.Bacc(
        "TRN2",
        target_bir_lowering=False,
        debug=False,
        enable_asserts=False,
    )
    idt_dt = {
        "i16": mybir.dt.int16,
        "i8": mybir.dt.int8,
        "pk2": mybir.dt.uint8,
        "pk2t": mybir.dt.uint8,
    }[IDX_MODE]
    idx_c = C // 4 if IDX_MODE in ("pk2", "pk2t") else C
    x_d = nc.dram_tensor(
        "x", [BPC, H, W, C], mybir.dt.float32, kind="ExternalInput"
    ).ap()
    i_d = nc.dram_tensor(
        "idx", [BPC, H, W, idx_c], idt_dt, kind="ExternalInput"
    ).ap()
    o_d = nc.dram_tensor(
        "out", [BPC, 2 * H, 2 * W, C], mybir.dt.float32, kind="ExternalOutput"
    ).ap()

    if layout == "bh":
        # Partition = (b, hh): whole-core tiles, 6 big clean DMAs, 12 DVE ops.
        assert IDX_MODE == "pk2t" and variant == "full"
        x_v2 = x_d.rearrange("b h w c -> (b h) (w c)")          # [128, 8192]
        i_v2 = i_d.rearrange("b h w cq -> (b h) (w cq)")        # [128, 2048]
        o_v2 = o_d.rearrange(
            "b (hh t) (sh wo) c -> (b hh) t sh (wo c)", t=2, sh=2
        )                                                       # [128,2,2,8192]
        FR = H * C  # 8192 free elements per partition (x side)
        with tile.TileContext(nc) as tc:
            with (
                tc.tile_pool(name="xp", bufs=2) as xp,
                tc.tile_pool(name="ip", bufs=2) as ip,
                tc.tile_pool(name="kp", bufs=2) as kp,
                tc.tile_pool(name="op", bufs=3) as op,
            ):
                for _rep in range(reps):
                    xt = xp.tile([128, FR], mybir.dt.float32)
                    pkt = ip.tile([128, FR // 4], mybir.dt.uint8)
                    nc.scalar.dma_start(xt[:], x_v2)
                    nc.scalar.dma_start(pkt[:], i_v2)
                    km = kp.tile([128, FR], mybir.dt.uint8)
                    km32 = (
                        km[:]
                        .bitcast(mybir.dt.uint32)
                        .rearrange("p (g y) -> p y g", y=4)
                    )
                    pk32 = pkt[:].bitcast(mybir.dt.uint32)
                    for y in range(4):
                        nc.vector.tensor_scalar(
                            km32[:, y],
                            pk32,
                            2 * y,
                            0x03030303,
                            mybir.AluOpType.logical_shift_right,
                            mybir.AluOpType.bitwise_and,
                        )
                    kmr = km[:].rearrange(
                        "p (sh wl c) -> p sh wl c", sh=2, c=C
                    )
                    xr = xt[:].rearrange(
                        "p (sh wl c) -> p sh wl c", sh=2, c=C
                    )
                    for t in (0, 1):
                        for sh in (0, 1):
                            ot = op.tile([128, FR], mybir.dt.float32)
                            ov = ot[:].rearrange(
                                "p (wl dw c) -> p wl dw c", dw=2, c=C
                            )
                            for dw in (0, 1):
                                nc.vector.scalar_tensor_tensor(
                                    out=ov[:, :, dw, :],
                                    in0=kmr[:, sh],
                                    scalar=t * 2 + dw,
                                    in1=xr[:, sh],
                                    op0=mybir.AluOpType.is_equal,
                                    op1=mybir.AluOpType.mult,
                                )
                            nc.sync.dma_start(o_v2[:, t, sh], ot[:])
        nc.compile()
        return nc

    # DRAM views. Input: partition p = (h, s), free = (wl c).
    x_v = x_d.rearrange("b h (s wl) c -> b h s (wl c)", s=S_)
    i_v = i_d.rearrange("b h (s wl) c -> b h s (wl c)", s=S_)
    FI = WL_ * idx_c  # free elements per partition on the idx side
    # Output: iter order (hh, s) = partitions, then free (t, wl, dw, c).
    o_v = o_d.rearrange(
        "b (hh t) (s wl dw) c -> b hh s t wl dw c", t=2, s=S_, wl=WL_, dw=2
    )

    op_t = mybir.AluOpType
    with tile.TileContext(nc) as tc:
        with (
            tc.tile_pool(name="xp", bufs=io_bufs) as xp,
            tc.tile_pool(name="ip", bufs=io_bufs) as ip,
            tc.tile_pool(name="kp", bufs=2) as kp,
            tc.tile_pool(name="mp", bufs=2) as mp,
            tc.tile_pool(name="op", bufs=op_bufs) as op,
        ):
            for it_r in range(reps * N_TILES_):
                it = it_r % N_TILES_
                b = it // TILES_PER_B_
                h0 = (it % TILES_PER_B_) * HC_

                xt = xp.tile([128, F_], mybir.dt.float32)
                idt = ip.tile([128, FI], idt_dt)
                if variant != "noindma":
                    if alt_rings:
                        ieng = (nc.scalar, nc.sync)[it % 2]
                    else:
                        ieng = nc.scalar if in_eng == "scalar" else nc.sync
                    ieng.dma_start(xt[:], x_v[b, h0 : h0 + HC_])
                    ieng.dma_start(idt[:], i_v[b, h0 : h0 + HC_])

                do_decode = variant in (
                    "full", "two_planes", "nooutdma", "noindma"
                )
                km_src = idt
                if IDX_MODE == "i16" and do_decode:
                    # koff-mask: keep bits 7 (dw) and 14 (dh).
                    km = kp.tile([128, F_], mybir.dt.int16)
                    and_eng = nc.gpsimd if AND_ENGINE == "g" else nc.vector
                    and_eng.tensor_scalar(
                        km[:], idt[:], 16512, None, op_t.bitwise_and
                    )
                    km_src = km
                elif IDX_MODE == "pk2t" and do_decode:
                    # one op per bit-pair y: (pk32 >> 2y) & 0x03030303 yields
                    # 4 codes per uint32, written natural-order at stride-4
                    # uint32 positions (offset y). All APs stay <= 3 dims.
                    km = kp.tile([128, F_], mybir.dt.uint8)
                    km32 = (
                        km[:]
                        .bitcast(mybir.dt.uint32)
                        .rearrange(
                            "p (wl g y) -> p y wl g", g=C // 16, y=4
                        )
                    )
                    pk32 = (
                        idt[:]
                        .bitcast(mybir.dt.uint32)
                        .rearrange("p (wl g) -> p wl g", g=C // 16)
                    )
                    for y in range(4):
                        nc.vector.tensor_scalar(
                            km32[:, y],
                            pk32,
                            2 * y,
                            0x03030303,
                            op_t.logical_shift_right,
                            op_t.bitwise_and,
                        )
                    km_src = km
                elif IDX_MODE == "pk2" and do_decode:
                    km = kp.tile([128, F_], mybir.dt.uint8)
                    if dec16:
                        # uint16 view: one op extracts TWO codes (bits 2r and
                        # 8+2r) -> halves decode element count. km is stored
                        # r-major within each c-block: byte = 32*(c%4) + c//4;
                        # the stt reads absorb the permutation in their AP.
                        km16 = (
                            km[:]
                            .bitcast(mybir.dt.uint16)
                            .rearrange(
                                "p (wl r j) -> p r wl j", r=4, j=C // 8
                            )
                        )
                        pk16 = (
                            idt[:]
                            .bitcast(mybir.dt.uint16)
                            .rearrange("p (wl j) -> p wl j", j=C // 8)
                        )
                        for r in range(4):
                            nc.vector.tensor_scalar(
                                km16[:, r],
                                pk16,
                                2 * r,
                                0x0303,
                                op_t.logical_shift_right,
                                op_t.bitwise_and,
                            )
                    else:
                        # km[.., 4q+r] = (pk[.., q] >> 2r) & 3
                        km_r4 = km[:].rearrange(
                            "p (wl q r) -> p r wl q", q=C // 4, r=4
                        )
                        pk_v = idt[:].rearrange(
                            "p (wl q) -> p wl q", q=C // 4
                        )
                        for r in range(4):
                            nc.vector.tensor_scalar(
                                km_r4[:, r],
                                pk_v,
                                2 * r,
                                3,
                                op_t.logical_shift_right,
                                op_t.bitwise_and,
                            )
                    km_src = km

                ot = op.tile([128, 4 * F_], mybir.dt.float32)
                ov = ot[:].rearrange(
                    "p (t wl dw c) -> p t wl dw c", t=2, wl=WL_, dw=2, c=C
                )
                planes = {
                    "full": 4, "noand": 4, "two_planes": 2,
                    "nodve": 0, "nooutdma": 4, "noindma": 4,
                }[variant]
                if planes and km_src[:].free_size() != F_:
                    planes = 0  # probe variant incompatible with pk2 decode
                use_perm = IDX_MODE == "pk2" and dec16 and do_decode
                if planes and use_perm:
                    # km is r-major (byte = 32*(c%4) + c//4); split c into
                    # (q, r) everywhere so free dims match across operands.
                    kmv = km_src[:].rearrange(
                        "p (wl r q) -> p wl q r", r=4, q=C // 4
                    )
                    xv = xt[:].rearrange(
                        "p (wl q r) -> p wl q r", q=C // 4, r=4
                    )
                    ov6 = ot[:].rearrange(
                        "p (t wl dw q r) -> p t wl dw q r",
                        t=2, wl=WL_, dw=2, q=C // 4, r=4,
                    )
                elif planes:
                    kmv = km_src[:].rearrange("p (wl c) -> p wl c", c=C)
                    xv = xt[:].rearrange("p (wl c) -> p wl c", c=C)
                if planes == 0:
                    # cheap writer so the out DMA has a producer
                    nc.vector.memset(ot[:], 0.0)
                m_dt = (
                    mybir.dt.bfloat16 if mask_dt == "bf16" else mybir.dt.float32
                )
                for dh in (0, 1):
                    for dw in (0, 1):
                        k_lin = dh * 2 + dw
                        if k_lin >= planes:
                            continue
                        cst = (
                            dh * 16384 + dw * 128
                            if IDX_MODE == "i16"
                            else k_lin
                        )
                        pl_out = (
                            ov6[:, dh, :, dw, :, :]
                            if use_perm
                            else ov[:, dh, :, dw, :]
                        )
                        if k_lin >= 4 - gps_planes:
                            # DVE builds a 0/1 mask (single-src ts, fast
                            # mode); the idle GPSIMD does the multiply.
                            mk = mp.tile([128, F_], m_dt)
                            nc.vector.tensor_scalar(
                                mk[:], km_src[:], cst, None, op_t.is_equal
                            )
                            nc.gpsimd.tensor_tensor(
                                pl_out,
                                mk[:].rearrange("p (wl c) -> p wl c", c=C),
                                xv,
                                op_t.mult,
                            )
                        else:
                            nc.vector.scalar_tensor_tensor(
                                out=pl_out,
                                in0=kmv,
                                scalar=cst,
                                in1=xv,
                                op0=op_t.is_equal,
                                op1=op_t.mult,
                            )

                # DMA APs allow at most 3 dims; split the store by t (=dh).
                if variant != "nooutdma":
                    for t in (0, 1):
                        if alt_rings:
                            oeng = (nc.sync, nc.scalar)[it % 2]
                        else:
                            oeng = (
                                nc.scalar
                                if (out_split and t == 1)
                                else nc.sync
                            )
                        oeng.dma_start(
                            o_v[b, h0 : h0 + HC_, :, t],
                            ot[:, t * 2 * F_ : (t + 1) * 2 * F_],
                        )

    nc.compile()
    return nc


def _get_program():
    if "nc" not in _CACHE:
        _CACHE["nc"] = build_program()
    return _CACHE["nc"]


def encode_index(index: np.ndarray) -> np.ndarray:
    if IDX_MODE == "i16":
        # Low 16 bits of the index keep bits 7 (dw) and 14 (dh).
        return np.ascontiguousarray(np.asarray(index).astype(np.int16))
    # 2-bit cell code koff = 2*dh + dw.
    idx = np.asarray(index)
    koff = (((idx >> 7) & 1) | ((idx >> 13) & 2)).astype(np.uint8)
    if IDX_MODE == "i8":
        return np.ascontiguousarray(koff.view(np.int8))
    if IDX_MODE == "pk2t":
        # byte 4g+d holds codes c = 16g+4y+d at bit-pair y
        k6 = koff.reshape(*koff.shape[:-1], koff.shape[-1] // 16, 4, 4)
        pk = (
            k6[..., 0, :]
            | (k6[..., 1, :] << 2)
            | (k6[..., 2, :] << 4)
            | (k6[..., 3, :] << 6)
        )
        return np.ascontiguousarray(pk.reshape(*koff.shape[:-1], -1))
    # pk2: pack 4 codes per byte along the channel axis
    k4 = koff.reshape(*koff.shape[:-1], koff.shape[-1] // 4, 4)
    pk = k4[..., 0] | (k4[..., 1] << 2) | (k4[..., 2] << 4) | (k4[..., 3] << 6)
    return np.ascontiguousarray(pk)


def shard_inputs(x: np.ndarray, index: np.ndarray):
    x = np.ascontiguousarray(np.asarray(x, dtype=np.float32))
    idx_e = encode_index(index)
    return [
        {
            "x": x[c * BPC : (c + 1) * BPC],
            "idx": idx_e[c * BPC : (c + 1) * BPC],
        }
        for c in range(N_CORES)
    ]


def kernel(x: np.ndarray, index: np.ndarray) -> np.ndarray:
    from concourse import bass_utils

    nc = _get_program()
    in_maps = shard_inputs(x, index)
    res = bass_utils.run_bass_kernel_spmd(nc, in_maps, core_ids=list(range(N_CORES)))
    return np.concatenate([r["out"] for r in res.results], axis=0)
